# revision 12
# baseline (speedup 1.0000x reference)
"""Trainium2 Bass kernel for nn_DecLayerJ (gnn message passing decoder layer).

Strategy (per NeuronCore, 8-way data parallel over B*N nodes):
  - Host pre-transposes h_E to feature-major fp8 [128, 3, TOK] so the device
    streams it straight into matmuls (no on-chip transpose, ~19 MB/core HBM).
  - Edge MLP: W1 as fp8 DoubleRow matmuls (k-tile pairs; odd tiles zero-padded
    via stride-0 broadcast), tanh-gelu on ACT, W2 in bf16 plus a rank-1
    fp8 DoubleRow term adding -16*(1-mask) to z2 so gelu emits masked h2
    directly (gelu(x-16) == 0).
  - K-sum: DVE add tree 48->24->12->6 (2x bf16 mode), then 6-way matmul
    accumulation into the node-phase W3 psum. Sum_k(mask) for the b3 term is
    computed on host and uploaded.
  - Node phase: residual in f32, FFN in bf16 with exact gelu (Win bias added
    via rank-1 matmuls so gelus batch in q-pairs), mask_V via rank-1 matmul
    broadcast; output stored feature-major, host transposes back.
  - Software-pipelined: ACT alternates h1(g) / h2(g-1) back-to-back; psum1
    and psum2 each hold 4 banks single-buffered.
"""

import os
import sys

for _p in ("/opt/trn_rl_repo", "/root/.axon_site/_ro/trn_rl_repo"):
    if os.path.isdir(_p) and _p not in sys.path:
        sys.path.insert(0, _p)

import numpy as np
import ml_dtypes
from contextlib import ExitStack

import concourse.bass as bass
import concourse.mybir as mybir
import concourse.tile as tile
from concourse import bacc
from concourse.bass_utils import run_bass_kernel_spmd

F32 = mybir.dt.float32
BF16 = mybir.dt.bfloat16
F8 = mybir.dt.float8e4
AF = mybir.ActivationFunctionType
DR = mybir.MatmulPerfMode.DoubleRow
ADD = mybir.AluOpType.add
MULT = mybir.AluOpType.mult

H = 128
C_E = 384
B, N, K = 2, 4096, 48
SCALE = 30.0
N_CORES = 8
NODES = B * N // N_CORES          # 1024 nodes per core
TOK = NODES * K                   # 49152 edge tokens per core
UN = 8                            # nodes per psum unit
UT = UN * K                       # 384 tokens per unit
GN = 32                           # nodes per group (4 units)
GT = GN * K                       # 1536 tokens per group
NG = NODES // GN                  # 32 groups
W1S = 32.0                        # fp8 W1 pre-scale
MBIG = 16.0                       # mask -> -inf shift for gelu zeroing

_CACHE = {}


def _build():
    nc = bacc.Bacc("TRN2", target_bir_lowering=False, debug=False)

    hE = nc.declare_dram_parameter("hE", [128, 3, TOK], F8, isOutput=False)
    hVT32 = nc.declare_dram_parameter("hVT32", [128, NODES], F32, isOutput=False)
    hVT8 = nc.declare_dram_parameter("hVT8", [128, NODES], F8, isOutput=False)
    mneg = nc.declare_dram_parameter("mneg", [1, TOK], F8, isOutput=False)
    msum = nc.declare_dram_parameter("msum", [1, NODES], BF16, isOutput=False)
    maskVbf = nc.declare_dram_parameter("maskVbf", [1, NODES], BF16, isOutput=False)
    f8pack = nc.declare_dram_parameter("f8pack", [128, 8, 128], F8, isOutput=False)
    bfpack = nc.declare_dram_parameter("bfpack", [128, 17, 128], BF16, isOutput=False)
    f32pack = nc.declare_dram_parameter("f32pack", [128, 8], F32, isOutput=False)
    ones512 = nc.declare_dram_parameter("ones512", [1, 512], BF16, isOutput=False)

    OUT = nc.declare_dram_parameter("OUT", [128, NODES], F32, isOutput=True)

    with tile.TileContext(nc) as tc, ExitStack() as ctx:
        wp = ctx.enter_context(tc.tile_pool(name="wp", bufs=1))
        acc = ctx.enter_context(tc.tile_pool(name="acc", bufs=1))

        # critical-path loads spread across HWDGE queues
        f8w = wp.tile([128, 8, 128], F8)
        nc.scalar.dma_start(out=f8w[:], in_=f8pack[:])
        vt8 = wp.tile([128, NODES], F8)
        nc.scalar.dma_start(out=vt8[:], in_=hVT8[:])
        f32w = wp.tile([128, 8], F32)
        nc.scalar.dma_start(out=f32w[:], in_=f32pack[:])
        bfw = wp.tile([128, 17, 128], BF16)
        nc.scalar.dma_start(out=bfw[:], in_=bfpack[:])
        mA = wp.tile([1, TOK], F8)
        # node-phase loads: emitted after the edge loop so their transfers
        # queue behind the h_E chunks on the serial DMA device
        ones5 = wp.tile([1, 512], BF16)
        vt32 = wp.tile([128, NODES], F32)
        msumw = wp.tile([1, NODES], BF16)
        mV = wp.tile([1, NODES], BF16)

        b1c = f32w[:, 0:1]
        b2c = f32w[:, 1:2]

        S6 = acc.tile([128, NODES, 6], BF16)
        hv1 = acc.tile([128, NODES], F32)
        hv1bf = acc.tile([128, NODES], BF16)
        outT = acc.tile([128, NODES], F32)

        # ---- edge phase: 32 groups of 32 nodes (1536 tokens, 4 psum units)
        with (
            tc.tile_pool(name="lp", bufs=3) as lp,
            tc.tile_pool(name="hp1", bufs=2) as hp1,
            tc.tile_pool(name="hp2", bufs=2) as hp2,
            tc.tile_pool(name="tp", bufs=2) as tp,
            tc.tile_pool(name="pp1", bufs=1, space="PSUM") as pp1,
            tc.tile_pool(name="pp2", bufs=1, space="PSUM") as pp2,
        ):
            def emit_stage2(st):
                h1t, n0, nu, tg = st
                t0 = n0 * K
                gt = nu * UT
                gn = nu * UN
                p2 = pp2.tile([128, 4, 512], F32)
                for u in range(nu):
                    tu = t0 + UT * u
                    nc.tensor.matmul(p2[:, u, :UT], bfw[:, 0, :],
                                     h1t[:, u, :], start=True, stop=False)
                    nc.tensor.matmul(
                        p2[:, u, :UT], f8w[0:1, 6:8, :],
                        mA[0:1, None, tu:tu + UT].to_broadcast([1, 2, UT]),
                        perf_mode=DR, start=False, stop=True)
                h2t = hp2.tile([128, nu, UT], BF16, tag=f"h2{tg}")
                nc.scalar.activation(h2t[:], p2[:, :nu, :UT],
                                     AF.Gelu_apprx_tanh, bias=b2c, scale=1.0)
                h2v = h2t[:].rearrange("p u (n k) -> p (u n) k", k=K)
                t24 = tp.tile([128, gn, 24], BF16, tag=f"t24{tg}")
                nc.vector.tensor_tensor(t24[:], h2v[:, :, 0:24],
                                        h2v[:, :, 24:48], ADD)
                t12 = tp.tile([128, gn, 12], BF16, tag=f"t12{tg}")
                nc.vector.tensor_tensor(t12[:], t24[:, :, 0:12],
                                        t24[:, :, 12:24], ADD)
                nc.vector.tensor_tensor(S6[:, n0:n0 + gn, :],
                                        t12[:, :, 0:6], t12[:, :, 6:12], ADD)

            # group schedule: two small prefill groups, then 4-unit groups
            groups = [(0, 1), (8, 3)] + [(32 * g, 4) for g in range(1, NG)]
            prev = None
            for gi, (n0, nu) in enumerate(groups):
                t0 = n0 * K
                gt = nu * UT
                tg = "" if nu == 4 else f"p{gi}"
                hEt = lp.tile([128, 3, gt], F8, tag=f"hE{tg}")
                nc.sync.dma_start(out=hEt[:], in_=hE[:, :, t0:t0 + gt])
                if gi == 0:
                    nc.sync.dma_start(out=mA[:], in_=mneg[:])

                psum1 = pp1.tile([128, 4, 512], F32)
                for u in range(nu):
                    tu = UT * u
                    nn = n0 + UN * u
                    nc.tensor.matmul(psum1[:, u, :UT], f8w[:, 0:2, :],
                                     hEt[:, 0:2, tu:tu + UT],
                                     perf_mode=DR, start=True, stop=False)
                    nc.tensor.matmul(
                        psum1[:, u, :UT], f8w[:, 2:4, :],
                        hEt[:, 2, None, tu:tu + UT].to_broadcast([128, 2, UT]),
                        perf_mode=DR, start=False, stop=False)
                    nc.tensor.matmul(
                        psum1[:, u, :UT], f8w[:, 4:6, :],
                        vt8[:, None, nn:nn + UN, None].to_broadcast(
                            [128, 2, UN, K]),
                        perf_mode=DR, start=False, stop=True)

                h1t = hp1.tile([128, nu, UT], BF16, tag=f"h1{tg}")
                nc.scalar.activation(h1t[:], psum1[:, :nu, :UT],
                                     AF.Gelu_apprx_tanh, bias=b1c,
                                     scale=1.0 / W1S)

                if gi == 25:
                    # node-phase data, queued behind the mid-stream h_E loads
                    nc.sync.dma_start(out=ones5[:], in_=ones512[:])
                    nc.sync.dma_start(out=vt32[:], in_=hVT32[:])
                    nc.sync.dma_start(out=msumw[:], in_=msum[:])
                    nc.sync.dma_start(out=mV[:], in_=maskVbf[:])

                if prev is not None:
                    emit_stage2(prev)
                prev = (h1t, n0, nu, tg)
            emit_stage2(prev)

        # ---- node phase (quarters of 256 nodes, pipelined)
        with (
            tc.tile_pool(name="np1", bufs=2, space="PSUM") as np1,
            tc.tile_pool(name="np2", bufs=1, space="PSUM") as np2,
            tc.tile_pool(name="np3", bufs=2, space="PSUM") as np3,
        ):
            for qn in (1, 2, 3, 0):
                sl = slice(256 * qn, 256 * (qn + 1))
                psum_dh = np1.tile([128, 256], F32)
                for k in range(6):
                    nc.tensor.matmul(psum_dh[:], bfw[:, 9, :],
                                     S6[:, sl, k], start=(k == 0), stop=False)
                nc.tensor.matmul(psum_dh[:], bfw[0:1, 10, :],
                                 msumw[0:1, sl], start=False, stop=True)
                nc.vector.tensor_tensor(hv1[:, sl], vt32[:, sl],
                                        psum_dh[:], ADD)
                nc.scalar.copy(hv1bf[:, sl], hv1[:, sl])

                psg = np2.tile([128, 4, 256], F32)
                for q in range(4):
                    nc.tensor.matmul(psg[:, q, :], bfw[:, 1 + q, :],
                                     hv1bf[:, sl], start=True, stop=False)
                    nc.tensor.matmul(psg[:, q, :], bfw[0:1, 13 + q, :],
                                     ones5[0:1, 0:256], start=False, stop=True)
                gq = acc.tile([128, 4, 256], BF16, tag="gq", bufs=2)
                nc.scalar.activation(gq[:], psg[:], AF.Gelu_apprx_tanh,
                                     bias=0.0, scale=1.0)
                pso = np3.tile([128, 256], F32, tag="pso")
                for q in range(4):
                    nc.tensor.matmul(pso[:], bfw[:, 5 + q, :],
                                     gq[:, q, :], start=(q == 0), stop=False)
                nc.tensor.matmul(pso[:], bfw[0:1, 11, :], ones5[0:1, 0:256],
                                 start=False, stop=True)
                psmv = np3.tile([128, 256], F32, tag="psmv")
                nc.tensor.matmul(psmv[:], bfw[0:1, 12, :], mV[0:1, sl],
                                 start=True, stop=True)
                o1 = acc.tile([128, 256], F32, tag="o1", bufs=2)
                nc.vector.tensor_tensor(o1[:], hv1[:, sl], pso[:], ADD)
                nc.vector.tensor_tensor(outT[:, sl], o1[:], psmv[:], MULT)
                nc.sync.dma_start(out=OUT[:, sl], in_=outT[:, sl])

    nc.compile()
    return nc


def _get_program():
    if "nc" not in _CACHE:
        _CACHE["nc"] = _build()
    return _CACHE["nc"]


def _prep_core_inputs(h_V, h_E, mask_V, mask_attend, W1_w, W1_b, W2_w, W2_b,
                      W3_w, W3_b, Win_w, Win_b, Wout_w, Wout_b):
    bf = ml_dtypes.bfloat16
    f8 = ml_dtypes.float8_e4m3

    f8pack = np.zeros((128, 8, 128), np.float32)
    W1e = np.asarray(W1_w, np.float32)[128:].reshape(3, 128, H)
    f8pack[:, 0, :] = W1e[0] * W1S
    f8pack[:, 1, :] = W1e[1] * W1S
    f8pack[:, 2, :] = W1e[2] * W1S
    f8pack[:, 4, :] = np.asarray(W1_w, np.float32)[:128] * W1S
    f8pack[0, 6, :] = MBIG
    f8pack = f8pack.astype(f8)

    bfpack = np.zeros((128, 17, 128), np.float32)
    bfpack[:, 0, :] = np.asarray(W2_w, np.float32)
    bfpack[:, 1:5, :] = np.asarray(Win_w, np.float32).reshape(128, 4, 128)
    bfpack[:, 5:9, :] = np.asarray(
        Wout_w, np.float32).reshape(4, 128, 128).transpose(1, 0, 2)
    bfpack[:, 9, :] = np.asarray(W3_w, np.float32) / SCALE
    bfpack[0, 10, :] = np.asarray(W3_b, np.float32) / SCALE
    bfpack[0, 11, :] = np.asarray(Wout_b, np.float32)
    bfpack[0, 12, :] = 1.0
    bfpack[0, 13:17, :] = np.asarray(Win_b, np.float32).reshape(4, 128)
    bfpack = bfpack.astype(bf)

    f32pack = np.zeros((128, 8), np.float32)
    f32pack[:, 0] = np.asarray(W1_b, np.float32)
    f32pack[:, 1] = np.asarray(W2_b, np.float32)

    shared = dict(
        f8pack=f8pack,
        bfpack=bfpack,
        f32pack=f32pack,
        ones512=np.ones((1, 512), bf),
    )

    hV_all = np.asarray(h_V, np.float32).reshape(B * N, H)
    hE_all = np.asarray(h_E, np.float32).reshape(B * N, K, C_E)
    mA_all = np.asarray(mask_attend, np.float32).reshape(B * N, K)
    mV_all = np.asarray(mask_V, np.float32).reshape(B * N)

    in_maps = []
    for i in range(N_CORES):
        s = slice(i * NODES, (i + 1) * NODES)
        hEc = np.ascontiguousarray(
            hE_all[s].reshape(TOK, C_E).T).reshape(3, 128, TOK)
        mAc = mA_all[s]
        in_maps.append(dict(
            hE=np.ascontiguousarray(hEc.transpose(1, 0, 2)).astype(f8),
            hVT32=np.ascontiguousarray(hV_all[s].T),
            hVT8=np.ascontiguousarray(hV_all[s].T).astype(f8),
            mneg=(mAc.reshape(1, TOK) - 1.0).astype(f8),
            msum=mAc.sum(axis=1).reshape(1, NODES).astype(bf),
            maskVbf=mV_all[s].reshape(1, NODES).astype(bf),
            **shared,
        ))
    return in_maps


def kernel(**inputs) -> np.ndarray:
    nc = _get_program()
    in_maps = _prep_core_inputs(**inputs)
    res = run_bass_kernel_spmd(nc, in_maps, list(range(N_CORES)))
    out = np.concatenate([np.asarray(r["OUT"], np.float32).T
                          for r in res.results], axis=0)
    return out.reshape(B, N, H)


# revision 13
# speedup vs baseline: 1.0237x; 1.0237x over previous
"""Trainium2 Bass kernel for nn_DecLayerJ (gnn message passing decoder layer).

Strategy (per NeuronCore, 8-way data parallel over B*N nodes):
  - Host pre-transposes h_E to feature-major fp8 [128, 3, TOK] so the device
    streams it straight into matmuls (no on-chip transpose, ~19 MB/core HBM).
  - Edge MLP: W1 as fp8 DoubleRow matmuls (k-tile pairs; odd tiles zero-padded
    via stride-0 broadcast), tanh-gelu on ACT, W2 in bf16 plus a rank-1
    fp8 DoubleRow term adding -16*(1-mask) to z2 so gelu emits masked h2
    directly (gelu(x-16) == 0).
  - K-sum: DVE add tree 48->24->12->6 (2x bf16 mode), then 6-way matmul
    accumulation into the node-phase W3 psum. Sum_k(mask) for the b3 term is
    computed on host and uploaded.
  - Node phase: residual in f32, FFN in bf16 with exact gelu (Win bias added
    via rank-1 matmuls so gelus batch in q-pairs), mask_V via rank-1 matmul
    broadcast; output stored feature-major, host transposes back.
  - Software-pipelined: ACT alternates h1(g) / h2(g-1) back-to-back; psum1
    and psum2 each hold 4 banks single-buffered.
"""

import os
import sys

for _p in ("/opt/trn_rl_repo", "/root/.axon_site/_ro/trn_rl_repo"):
    if os.path.isdir(_p) and _p not in sys.path:
        sys.path.insert(0, _p)

import numpy as np
import ml_dtypes
from contextlib import ExitStack

import concourse.bass as bass
import concourse.mybir as mybir
import concourse.tile as tile
from concourse import bacc
from concourse.bass_utils import run_bass_kernel_spmd

F32 = mybir.dt.float32
BF16 = mybir.dt.bfloat16
F8 = mybir.dt.float8e4
AF = mybir.ActivationFunctionType
DR = mybir.MatmulPerfMode.DoubleRow
ADD = mybir.AluOpType.add
MULT = mybir.AluOpType.mult

H = 128
C_E = 384
B, N, K = 2, 4096, 48
SCALE = 30.0
N_CORES = 8
NODES = B * N // N_CORES          # 1024 nodes per core
TOK = NODES * K                   # 49152 edge tokens per core
UN = 8                            # nodes per psum unit
UT = UN * K                       # 384 tokens per unit
GN = 32                           # nodes per group (4 units)
GT = GN * K                       # 1536 tokens per group
NG = NODES // GN                  # 32 groups
W1S = 32.0                        # fp8 W1 pre-scale
MBIG = 16.0                       # mask -> -inf shift for gelu zeroing

_CACHE = {}


def _build():
    nc = bacc.Bacc("TRN2", target_bir_lowering=False, debug=False)

    hE = nc.declare_dram_parameter("hE", [128, 3, TOK], F8, isOutput=False)
    hVT32 = nc.declare_dram_parameter("hVT32", [128, NODES], F32, isOutput=False)
    hVT8 = nc.declare_dram_parameter("hVT8", [128, NODES], F8, isOutput=False)
    mneg = nc.declare_dram_parameter("mneg", [1, TOK], F8, isOutput=False)
    msum = nc.declare_dram_parameter("msum", [1, NODES], BF16, isOutput=False)
    maskVbf = nc.declare_dram_parameter("maskVbf", [1, NODES], BF16, isOutput=False)
    f8pack = nc.declare_dram_parameter("f8pack", [128, 8, 128], F8, isOutput=False)
    bfpack = nc.declare_dram_parameter("bfpack", [128, 17, 128], BF16, isOutput=False)
    f32pack = nc.declare_dram_parameter("f32pack", [128, 8], F32, isOutput=False)
    ones512 = nc.declare_dram_parameter("ones512", [1, 512], BF16, isOutput=False)

    OUT = nc.declare_dram_parameter("OUT", [128, NODES], F32, isOutput=True)

    with tile.TileContext(nc) as tc, ExitStack() as ctx:
        wp = ctx.enter_context(tc.tile_pool(name="wp", bufs=1))
        acc = ctx.enter_context(tc.tile_pool(name="acc", bufs=1))

        # critical-path loads spread across HWDGE queues
        f8w = wp.tile([128, 8, 128], F8)
        nc.scalar.dma_start(out=f8w[:], in_=f8pack[:])
        vt8 = wp.tile([128, NODES], F8)
        nc.scalar.dma_start(out=vt8[:], in_=hVT8[:])
        f32w = wp.tile([128, 8], F32)
        nc.scalar.dma_start(out=f32w[:], in_=f32pack[:])
        bfw = wp.tile([128, 17, 128], BF16)
        nc.scalar.dma_start(out=bfw[:], in_=bfpack[:])
        mA = wp.tile([1, TOK], F8)
        # node-phase loads: emitted after the edge loop so their transfers
        # queue behind the h_E chunks on the serial DMA device
        ones5 = wp.tile([1, 512], BF16)
        vt32 = wp.tile([128, NODES], F32)
        msumw = wp.tile([1, NODES], BF16)
        mV = wp.tile([1, NODES], BF16)

        b1c = f32w[:, 0:1]
        b2c = f32w[:, 1:2]

        S6 = acc.tile([128, NODES, 6], BF16)
        hv1 = acc.tile([128, NODES], F32)
        hv1bf = acc.tile([128, NODES], BF16)
        outT = acc.tile([128, NODES], F32)

        # ---- edge phase: 32 groups of 32 nodes (1536 tokens, 4 psum units)
        with (
            tc.tile_pool(name="lp", bufs=3) as lp,
            tc.tile_pool(name="hp1", bufs=2) as hp1,
            tc.tile_pool(name="hp2", bufs=2) as hp2,
            tc.tile_pool(name="tp", bufs=2) as tp,
            tc.tile_pool(name="pp1", bufs=1, space="PSUM") as pp1,
            tc.tile_pool(name="pp2", bufs=1, space="PSUM") as pp2,
        ):
            def emit_stage2(st):
                h1t, n0, nu, tg = st
                t0 = n0 * K
                gt = nu * UT
                gn = nu * UN
                p2 = pp2.tile([128, 4, 512], F32)
                for u in range(nu):
                    tu = t0 + UT * u
                    nc.tensor.matmul(p2[:, u, :UT], bfw[:, 0, :],
                                     h1t[:, u, :], start=True, stop=False)
                    nc.tensor.matmul(
                        p2[:, u, :UT], f8w[0:1, 6:8, :],
                        mA[0:1, None, tu:tu + UT].to_broadcast([1, 2, UT]),
                        perf_mode=DR, start=False, stop=True)
                h2t = hp2.tile([128, nu, UT], BF16, tag=f"h2{tg}")
                nc.scalar.activation(h2t[:], p2[:, :nu, :UT],
                                     AF.Gelu_apprx_tanh, bias=b2c, scale=1.0)
                h2v = h2t[:].rearrange("p u (n k) -> p (u n) k", k=K)
                t24 = tp.tile([128, gn, 24], BF16, tag=f"t24{tg}")
                nc.vector.tensor_tensor(t24[:], h2v[:, :, 0:24],
                                        h2v[:, :, 24:48], ADD)
                t12 = tp.tile([128, gn, 12], BF16, tag=f"t12{tg}")
                nc.vector.tensor_tensor(t12[:], t24[:, :, 0:12],
                                        t24[:, :, 12:24], ADD)
                nc.vector.tensor_tensor(S6[:, n0:n0 + gn, :],
                                        t12[:, :, 0:6], t12[:, :, 6:12], ADD)

            groups = [(32 * g, 4) for g in range(NG)]
            prev = None
            for gi, (n0, nu) in enumerate(groups):
                t0 = n0 * K
                gt = nu * UT
                tg = "" if nu == 4 else f"p{gi}"
                hEt = lp.tile([128, 3, gt], F8, tag=f"hE{tg}")
                nc.sync.dma_start(out=hEt[:], in_=hE[:, :, t0:t0 + gt])
                if gi == 0:
                    nc.sync.dma_start(out=mA[:], in_=mneg[:])

                psum1 = pp1.tile([128, 4, 512], F32)
                for u in range(nu):
                    tu = UT * u
                    nn = n0 + UN * u
                    nc.tensor.matmul(psum1[:, u, :UT], f8w[:, 0:2, :],
                                     hEt[:, 0:2, tu:tu + UT],
                                     perf_mode=DR, start=True, stop=False)
                    nc.tensor.matmul(
                        psum1[:, u, :UT], f8w[:, 2:4, :],
                        hEt[:, 2, None, tu:tu + UT].to_broadcast([128, 2, UT]),
                        perf_mode=DR, start=False, stop=False)
                    nc.tensor.matmul(
                        psum1[:, u, :UT], f8w[:, 4:6, :],
                        vt8[:, None, nn:nn + UN, None].to_broadcast(
                            [128, 2, UN, K]),
                        perf_mode=DR, start=False, stop=True)

                h1t = hp1.tile([128, nu, UT], BF16, tag=f"h1{tg}")
                nc.scalar.activation(h1t[:], psum1[:, :nu, :UT],
                                     AF.Gelu_apprx_tanh, bias=b1c,
                                     scale=1.0 / W1S)

                if gi == 25:
                    # node-phase data, queued behind the mid-stream h_E loads
                    nc.sync.dma_start(out=ones5[:], in_=ones512[:])
                    nc.sync.dma_start(out=vt32[:], in_=hVT32[:])
                    nc.sync.dma_start(out=msumw[:], in_=msum[:])
                    nc.sync.dma_start(out=mV[:], in_=maskVbf[:])

                if prev is not None:
                    emit_stage2(prev)
                prev = (h1t, n0, nu, tg)
            emit_stage2(prev)

        # ---- node phase (quarters of 256 nodes, pipelined)
        with (
            tc.tile_pool(name="np1", bufs=2, space="PSUM") as np1,
            tc.tile_pool(name="np2", bufs=1, space="PSUM") as np2,
            tc.tile_pool(name="np3", bufs=2, space="PSUM") as np3,
        ):
            for qn in (1, 2, 3, 0):
                sl = slice(256 * qn, 256 * (qn + 1))
                psum_dh = np1.tile([128, 256], F32)
                for k in range(6):
                    nc.tensor.matmul(psum_dh[:], bfw[:, 9, :],
                                     S6[:, sl, k], start=(k == 0), stop=False)
                nc.tensor.matmul(psum_dh[:], bfw[0:1, 10, :],
                                 msumw[0:1, sl], start=False, stop=True)
                nc.vector.tensor_tensor(hv1[:, sl], vt32[:, sl],
                                        psum_dh[:], ADD)
                nc.scalar.copy(hv1bf[:, sl], hv1[:, sl])

                psg = np2.tile([128, 4, 256], F32)
                for q in range(4):
                    nc.tensor.matmul(psg[:, q, :], bfw[:, 1 + q, :],
                                     hv1bf[:, sl], start=True, stop=False)
                    nc.tensor.matmul(psg[:, q, :], bfw[0:1, 13 + q, :],
                                     ones5[0:1, 0:256], start=False, stop=True)
                gq = acc.tile([128, 4, 256], BF16, tag="gq", bufs=2)
                nc.scalar.activation(gq[:], psg[:], AF.Gelu_apprx_tanh,
                                     bias=0.0, scale=1.0)
                pso = np3.tile([128, 256], F32, tag="pso")
                for q in range(4):
                    nc.tensor.matmul(pso[:], bfw[:, 5 + q, :],
                                     gq[:, q, :], start=(q == 0), stop=False)
                nc.tensor.matmul(pso[:], bfw[0:1, 11, :], ones5[0:1, 0:256],
                                 start=False, stop=True)
                psmv = np3.tile([128, 256], F32, tag="psmv")
                nc.tensor.matmul(psmv[:], bfw[0:1, 12, :], mV[0:1, sl],
                                 start=True, stop=True)
                o1 = acc.tile([128, 256], F32, tag="o1", bufs=2)
                nc.vector.tensor_tensor(o1[:], hv1[:, sl], pso[:], ADD)
                nc.vector.tensor_tensor(outT[:, sl], o1[:], psmv[:], MULT)
                nc.sync.dma_start(out=OUT[:, sl], in_=outT[:, sl])

    nc.compile()
    return nc


def _get_program():
    if "nc" not in _CACHE:
        _CACHE["nc"] = _build()
    return _CACHE["nc"]


def _prep_core_inputs(h_V, h_E, mask_V, mask_attend, W1_w, W1_b, W2_w, W2_b,
                      W3_w, W3_b, Win_w, Win_b, Wout_w, Wout_b):
    bf = ml_dtypes.bfloat16
    f8 = ml_dtypes.float8_e4m3

    f8pack = np.zeros((128, 8, 128), np.float32)
    W1e = np.asarray(W1_w, np.float32)[128:].reshape(3, 128, H)
    f8pack[:, 0, :] = W1e[0] * W1S
    f8pack[:, 1, :] = W1e[1] * W1S
    f8pack[:, 2, :] = W1e[2] * W1S
    f8pack[:, 4, :] = np.asarray(W1_w, np.float32)[:128] * W1S
    f8pack[0, 6, :] = MBIG
    f8pack = f8pack.astype(f8)

    bfpack = np.zeros((128, 17, 128), np.float32)
    bfpack[:, 0, :] = np.asarray(W2_w, np.float32)
    bfpack[:, 1:5, :] = np.asarray(Win_w, np.float32).reshape(128, 4, 128)
    bfpack[:, 5:9, :] = np.asarray(
        Wout_w, np.float32).reshape(4, 128, 128).transpose(1, 0, 2)
    bfpack[:, 9, :] = np.asarray(W3_w, np.float32) / SCALE
    bfpack[0, 10, :] = np.asarray(W3_b, np.float32) / SCALE
    bfpack[0, 11, :] = np.asarray(Wout_b, np.float32)
    bfpack[0, 12, :] = 1.0
    bfpack[0, 13:17, :] = np.asarray(Win_b, np.float32).reshape(4, 128)
    bfpack = bfpack.astype(bf)

    f32pack = np.zeros((128, 8), np.float32)
    f32pack[:, 0] = np.asarray(W1_b, np.float32)
    f32pack[:, 1] = np.asarray(W2_b, np.float32)

    shared = dict(
        f8pack=f8pack,
        bfpack=bfpack,
        f32pack=f32pack,
        ones512=np.ones((1, 512), bf),
    )

    hV_all = np.asarray(h_V, np.float32).reshape(B * N, H)
    hE_all = np.asarray(h_E, np.float32).reshape(B * N, K, C_E)
    mA_all = np.asarray(mask_attend, np.float32).reshape(B * N, K)
    mV_all = np.asarray(mask_V, np.float32).reshape(B * N)

    in_maps = []
    for i in range(N_CORES):
        s = slice(i * NODES, (i + 1) * NODES)
        hEc = np.ascontiguousarray(
            hE_all[s].reshape(TOK, C_E).T).reshape(3, 128, TOK)
        mAc = mA_all[s]
        in_maps.append(dict(
            hE=np.ascontiguousarray(hEc.transpose(1, 0, 2)).astype(f8),
            hVT32=np.ascontiguousarray(hV_all[s].T),
            hVT8=np.ascontiguousarray(hV_all[s].T).astype(f8),
            mneg=(mAc.reshape(1, TOK) - 1.0).astype(f8),
            msum=mAc.sum(axis=1).reshape(1, NODES).astype(bf),
            maskVbf=mV_all[s].reshape(1, NODES).astype(bf),
            **shared,
        ))
    return in_maps


def kernel(**inputs) -> np.ndarray:
    nc = _get_program()
    in_maps = _prep_core_inputs(**inputs)
    res = run_bass_kernel_spmd(nc, in_maps, list(range(N_CORES)))
    out = np.concatenate([np.asarray(r["OUT"], np.float32).T
                          for r in res.results], axis=0)
    return out.reshape(B, N, H)
